# revision 23
# baseline (speedup 1.0000x reference)
"""SO3Conv Trainium2 Bass kernel.

Math (per reference):
  psi[f,g,i] = sum_n D[n,i] w[f,g,n] / sqrt(64)
  per l (d=2l+1, blk=d*d at offset off):
    y[b,g,off+v*d+m] = 1/sqrt(64*d) * sum_{f,u} x[b,f,off+u*d+m] * psi[f,g,off+u*d+v]

Strategy: data-parallel over batch (8 cores x 128 batch).
Per core:
  A) x is pre-permuted on the host into per-l regions [b, (m, u-pad, f)]
     bf16 (u padded to d+1 slots).  XBAR DMA-transposes (InstDmaTransposeAnt)
     produce the matmul lhsT tiles [(u,f)-part, b-free] directly from DRAM.
  B) wT [n, (f g)] via two XBARs from host-padded w2 [(f g), n-pad].
  C) psi computed on PE in psiT layout [i-chunk-part, (f g)-free] (D
     pre-scaled per l on device), parked in DRAM scratch (one tensor per
     chunk), read back into per-l rhs tiles [(u-pair,f)-part, ku:(v,g)-free]
     -- per-u for l6 (fine-grained early feed), per-u-parity for l<6.
  D) main matmuls run ku-outer over m-groups (8 PSUM banks) so the PE
     consumes psi readbacks as they stream in; PSUM [b,(v g)] fp32 copied
     (cast bf16) into per-l y tiles in natural [b, g, v*d+m] order, stored
     bf16 to per-l DRAM regions; host converts to fp32 and reassembles.
  DMA queues: sync carries the latency-critical chain in FIFO priority order
  (D, wT, x-l6, psi parks + readbacks, remaining x); scalar carries y stores.
"""

import sys

sys.path.insert(0, "/opt/trn_rl_repo")

import numpy as np

LMAX = 6
F = 64
NROT = 64
IRREP = 455
B = 1024
NCORES = 8
BS = B // NCORES  # 128

DS = [2 * l + 1 for l in range(LMAX + 1)]
OFFS = []
_o = 0
for _d in DS:
    OFFS.append(_o)
    _o += _d * _d
assert _o == IRREP

LORDER = list(range(LMAX, -1, -1))  # process l descending

# x4 DRAM region offsets (l descending), cols per l = d*(d+1)*64
XLEN = {l: DS[l] * (DS[l] + 1) * 64 for l in LORDER}
XOFF = {}
_o = 0
for l in LORDER:
    XOFF[l] = _o
    _o += XLEN[l]
XTOT = _o  # 32256

# y DRAM region offsets (l descending), cols per l = 64*blk
YLEN = {l: 64 * DS[l] * DS[l] for l in LORDER}
YOFF = {}
_o = 0
for l in LORDER:
    YOFF[l] = _o
    _o += YLEN[l]
YTOT = _o  # 29120

# psi matmul chunks: contiguous i-ranges, <=128 rows, l=6 first; one DRAM
# scratch tensor per chunk.
PSI_CHUNKS = [
    (OFFS[6], OFFS[6] + 9 * 13),       # c0: l6 u0..8   (117 rows)
    (OFFS[6] + 9 * 13, IRREP),         # c1: l6 u9..12  (52 rows)
    (OFFS[5], OFFS[6]),                # c2: l5         (121 rows)
    (OFFS[4], OFFS[5]),                # c3: l4         (81 rows)
    (0, OFFS[4]),                      # c4: l0..l3     (84 rows)
]

_CACHE = {}


def _build():
    import concourse.bacc as bacc
    import concourse.bass as bass
    import concourse.mybir as mybir
    from concourse import tile

    dt = mybir.dt
    BF = dt.bfloat16
    F32 = dt.float32

    nc = bacc.Bacc("TRN2", target_bir_lowering=False, debug=False, num_devices=NCORES)

    x_d = nc.dram_tensor("x4", [BS, XTOT], BF, kind="ExternalInput")
    w_d = nc.dram_tensor("w2", [F * F, 128], BF, kind="ExternalInput")
    D_d = nc.dram_tensor("D", [NROT, IRREP], F32, kind="ExternalInput")
    y_d = nc.dram_tensor("y", [BS, YTOT], BF, kind="ExternalOutput")
    # rows padded so rb_par's "(i2 j)" split (j=2d) divides evenly for every
    # l read from the chunk; pad rows are never written or read.
    PADROWS = {0: 117, 1: 52, 2: 132, 3: 90, 4: 210}
    park_t = [
        nc.dram_tensor(f"psiS{ci}", [PADROWS[ci], F * F], BF)
        for ci in range(len(PSI_CHUNKS))
    ]

    eng_flip = [0]

    with tile.TileContext(nc) as tc:
        with (
            tc.tile_pool(name="const", bufs=1) as cp,
            tc.tile_pool(name="xt", bufs=1) as xp,
            tc.tile_pool(name="rhs", bufs=1) as rp,
            tc.tile_pool(name="yb", bufs=1) as yp,
            tc.tile_pool(name="psit", bufs=4) as psp,
        ):
            # ---- persistent tiles ----
            wT = cp.tile([128, F * F], BF)
            d_f32 = cp.tile([NROT, IRREP], F32)
            d_pre = cp.tile([NROT, IRREP], BF)
            xt = {}   # l>=4: [128, nchunk, 128]; 'sm' = l3..l0 combined
            rhs = {}  # per l: [128, nku*d*64]; ku slab cols [ku*d*64, ...)
            yb = {}   # l>=4 per l; 'sm' combined for l3..l0
            for l in (6, 5, 4):
                d = DS[l]
                xt[l] = xp.tile(
                    [128, d * (d + 1) // 2, 128], BF, name=f"xt{l}", tag=f"xt{l}"
                )
                yb[l] = yp.tile([BS, 64 * d * d], BF, name=f"yb{l}", tag=f"yb{l}")
            NSM = sum(DS[l] * (DS[l] + 1) // 2 for l in (3, 2, 1, 0))  # 50
            xt["sm"] = xp.tile([128, NSM, 128], BF, name="xtsm", tag="xtsm")
            CB = {}  # chunk base within xt['sm']
            _c = 0
            for l in (3, 2, 1, 0):
                CB[l] = _c
                _c += DS[l] * (DS[l] + 1) // 2
            YSM = sum(YLEN[l] for l in (3, 2, 1, 0))  # 5376
            yb["sm"] = yp.tile([BS, YSM], BF, name="ybsm", tag="ybsm")
            YB = {l: YOFF[l] - YOFF[3] for l in (3, 2, 1, 0)}
            for l in LORDER:
                d = DS[l]
                rhs[l] = rp.tile(
                    [128, ((d + 1) // 2) * d * 64], BF, name=f"rhs{l}", tag=f"rhs{l}"
                )

            # ---- emission helpers ----
            def xbar(l, c0, c1, q=None):
                t = xt[l] if l in xt else xt["sm"]
                (q or nc.sync).dma_start(
                    t[:, c0:c1, :],
                    x_d[:, XOFF[l] + c0 * 128 : XOFF[l] + c1 * 128]
                    if l != "sm"
                    else x_d[:, XOFF[3] + c0 * 128 : XOFF[3] + c1 * 128],
                    transpose=True,
                )

            psiT_tiles = {}

            def park(ci, q=None, rows=None):
                r0, r1 = PSI_CHUNKS[ci]
                a, b = rows if rows is not None else (0, r1 - r0)
                (q or nc.sync).dma_start(
                    park_t[ci][a:b, :], psiT_tiles[ci][a:b, :]
                )

            def rb_u(l, u, q=None):
                """Per-u readback."""
                d = DS[l]
                ur0 = OFFS[l] + u * d
                for ci, (r0, r1) in enumerate(PSI_CHUNKS):
                    if r0 <= ur0 and ur0 + d <= r1:
                        break
                ku, uin = divmod(u, 2)
                dst = rhs[l][
                    uin * 64 : (uin + 1) * 64,
                    ku * d * 64 : (ku + 1) * d * 64,
                ].rearrange("f (v g) -> f v g", g=64)
                sv = park_t[ci].rearrange("i (f g) -> f i g", g=64)
                (q or nc.sync).dma_start(dst, sv[:, ur0 - r0 : ur0 - r0 + d, :])

            def rb_par(l, uin):
                """Per-u-parity readback: all ku slabs of one parity at once."""
                d = DS[l]
                nk = (d - uin + 1) // 2  # number of u's with this parity
                u0row = OFFS[l] + uin * d  # first row of u=uin
                for ci, (r0, r1) in enumerate(PSI_CHUNKS):
                    if r0 <= u0row and OFFS[l] + d * d <= r1:
                        break
                else:
                    raise AssertionError((l, uin))
                dst = rhs[l][
                    uin * 64 : (uin + 1) * 64, : nk * d * 64
                ].rearrange("f (ku v g) -> f ku v g", ku=nk, g=64)
                sv = park_t[ci].rearrange("i (f g) -> f i g", g=64).rearrange(
                    "f (i2 j) g -> f i2 j g", j=2 * d
                )
                # rows for parity uin, ku: (2*ku+uin)*d + v  = ku*(2d) + uin*d + v
                base = u0row - r0
                i2_0 = base // (2 * d)
                voff = base % (2 * d)
                src = sv[:, i2_0 : i2_0 + nk, voff : voff + d, :]
                nc.sync.dma_start(dst, src)

            # sync queue, FIFO priority order:
            nc.sync.dma_start(wT[:, : 2048], w_d[:2048, :], transpose=True)
            nc.sync.dma_start(d_f32[:, :], D_d[:, :])
            nc.sync.dma_start(wT[:, 2048:], w_d[2048:, :], transpose=True)
            for l in LORDER:
                off, blk = OFFS[l], DS[l] * DS[l]
                nc.scalar.mul(
                    d_pre[:, off : off + blk],
                    d_f32[:, off : off + blk],
                    1.0 / (64.0 * np.sqrt(DS[l])),
                )

            def psi_mm(ci, pa):
                r0, r1 = PSI_CHUNKS[ci]
                rows = r1 - r0
                psiT = psp.tile([128, F * F], BF, tag="psiT", name=f"psiT{ci}")
                psiT_tiles[ci] = psiT
                for p in range(4):
                    pps = pa.tile([128, 1024], F32, tag="pps", name=f"pps{ci}_{p}")
                    for h in range(2):
                        s = 2 * p + h
                        nc.tensor.matmul(
                            pps[:rows, h * 512 : (h + 1) * 512],
                            d_pre[:, r0:r1],
                            wT[:NROT, s * 512 : (s + 1) * 512],
                            start=True,
                            stop=True,
                        )
                    dst = psiT[:rows, p * 1024 : (p + 1) * 1024]
                    if eng_flip[0] % 2 == 0:
                        nc.vector.tensor_copy(dst, pps[:rows, :])
                    else:
                        nc.scalar.copy(dst, pps[:rows, :])
                    eng_flip[0] += 1

            # ---- orchestrated prologue ----
            with tc.tile_pool(
                name="pa", bufs=4, space=bass.MemorySpace.PSUM
            ) as pa:
                # sync (SP): l6 chain
                xbar(6, 0, 56)            # l6 m0-7
                psi_mm(0, pa)
                park(0)
                for u in range(0, 9):
                    rb_u(6, u)
                psi_mm(1, pa)
                park(1)
                for u in range(9, 13):
                    rb_u(6, u)
                xbar(6, 56, 91)           # l6 m8-12
                psi_mm(2, pa)
                # sync continues: l5 chain (park2 waits psi copies, so the
                # x5/x4/xsm XBARs queue behind the l6-critical prologue)
                park(2)
                for u in range(11):
                    rb_u(5, u)
                psi_mm(3, pa)
                # gpsimd (Pool SWDGE, otherwise idle): l4 + l3..l0 chains
                park(3, nc.gpsimd)
                for u in range(9):
                    rb_u(4, u, nc.gpsimd)
                xbar(5, 0, 48)            # l5 m0-7
                xbar(5, 48, 66)
                xbar(4, 0, 45)
                psi_mm(4, pa)
                park(4, nc.gpsimd)
                for l in (3, 2, 1, 0):
                    for u in range(DS[l]):
                        rb_u(l, u, nc.gpsimd)
                xbar("sm", 0, NSM)

            # ---- main loop ----
            with tc.tile_pool(
                name="py", bufs=1, space=bass.MemorySpace.PSUM
            ) as py:
                for l in LORDER:
                    d = DS[l]
                    nku = (d + 1) // 2
                    if d * 64 <= 512:
                        vsplits = [(0, d)]
                    else:
                        vsplits = [(0, 8), (8, d - 8)]
                    mg_size = 4 if len(vsplits) == 2 else 8
                    xtile = xt[l] if l >= 4 else xt["sm"]
                    cbase = 0 if l >= 4 else CB[l]
                    ytile = yb[l] if l >= 4 else yb["sm"]
                    ybase = 0 if l >= 4 else YB[l]
                    yv = ytile[:, ybase : ybase + 64 * d * d].rearrange(
                        "b (g v m) -> b g v m", g=64, v=d
                    )
                    for mg0 in range(0, d, mg_size):
                        ms = list(range(mg0, min(d, mg0 + mg_size)))
                        pyt = {}
                        for m in ms:
                            for vi, (v0, nv) in enumerate(vsplits):
                                slot = (m - mg0) * len(vsplits) + vi
                                pyt[(m, v0)] = py.tile(
                                    [BS, 512], F32, tag=f"py{slot}",
                                    name=f"py{l}_{m}_{v0}",
                                )
                        for ku in range(nku):
                            kk = 64 if (2 * ku + 1) >= d else 128
                            for m in ms:
                                c = cbase + m * nku + ku
                                for (v0, nv) in vsplits:
                                    nc.tensor.matmul(
                                        pyt[(m, v0)][:, : nv * 64],
                                        xtile[:kk, c, :],
                                        rhs[l][
                                            :kk,
                                            ku * d * 64
                                            + v0 * 64 : ku * d * 64
                                            + (v0 + nv) * 64,
                                        ],
                                        start=(ku == 0),
                                        stop=(ku == nku - 1),
                                    )
                        for m in ms:
                            for (v0, nv) in vsplits:
                                dst = yv[:, :, v0 : v0 + nv, m]
                                src = pyt[(m, v0)][:, : nv * 64].rearrange(
                                    "b (v g) -> b g v", g=64
                                )
                                if eng_flip[0] % 2 == 0:
                                    nc.scalar.copy(dst, src)
                                else:
                                    nc.vector.tensor_copy(dst, src)
                                eng_flip[0] += 1
                    if l >= 4:
                        (nc.sync if l == 6 else nc.gpsimd).dma_start(
                            y_d[:, YOFF[l] : YOFF[l] + YLEN[l]], yb[l][:, :]
                        )
                    elif l == 2:
                        cut = YB[1]
                        nc.gpsimd.dma_start(
                            y_d[:, YOFF[3] : YOFF[3] + cut], yb["sm"][:, :cut]
                        )
                    elif l == 0:
                        cut = YB[1]
                        nc.gpsimd.dma_start(
                            y_d[:, YOFF[3] + cut : YOFF[3] + YSM],
                            yb["sm"][:, cut:],
                        )

    nc.compile()
    return nc


def _get_nc():
    if "nc" not in _CACHE:
        _CACHE["nc"] = _build()
    return _CACHE["nc"]


def _prep_x(xc):
    """[BS, F, IRREP] fp32 -> [BS, XTOT] bf16 in per-l (m, u-pad, f) layout."""
    import ml_dtypes

    out = np.zeros((BS, XTOT), dtype=ml_dtypes.bfloat16)
    for l in LORDER:
        d = DS[l]
        off = OFFS[l]
        xl = xc[:, :, off : off + d * d].reshape(BS, F, d, d)  # [b, f, u, m]
        arr = np.zeros((BS, d, d + 1, F), dtype=np.float32)  # [b, m, u-pad, f]
        arr[:, :, :d, :] = xl.transpose(0, 3, 2, 1)
        out[:, XOFF[l] : XOFF[l] + XLEN[l]] = (
            arr.reshape(BS, XLEN[l]).astype(ml_dtypes.bfloat16)
        )
    return out


def kernel(x, D, w):
    import ml_dtypes
    from concourse.bass_utils import run_bass_kernel_spmd

    nc = _get_nc()
    w2 = np.zeros((F * F, 128), dtype=ml_dtypes.bfloat16)
    w2[:, :NROT] = (
        np.asarray(w, dtype=np.float32)
        .reshape(F * F, NROT)
        .astype(ml_dtypes.bfloat16)
    )
    Dc = np.ascontiguousarray(np.asarray(D, dtype=np.float32))
    in_maps = [
        {
            "x4": _prep_x(np.asarray(x[c * BS : (c + 1) * BS], dtype=np.float32)),
            "w2": w2,
            "D": Dc,
        }
        for c in range(NCORES)
    ]
    res = run_bass_kernel_spmd(nc, in_maps, core_ids=list(range(NCORES)))
    yflat = np.concatenate(
        [r["y"].astype(np.float32) for r in res.results], axis=0
    )  # [B, YTOT]
    y = np.empty((B, F, IRREP), dtype=np.float32)
    for l in LORDER:
        d = DS[l]
        blk = d * d
        y[:, :, OFFS[l] : OFFS[l] + blk] = yflat[
            :, YOFF[l] : YOFF[l] + YLEN[l]
        ].reshape(B, F, blk)
    return y
